# revision 2
# baseline (speedup 1.0000x reference)
"""BatchAllTripletLoss kernel for Trainium2 (8 NeuronCores, Bass/Tile).

Math shortcut: with labels = [0..N-1, 0..N-1], the (positive, negative)
mask of the [2N,2N,2N] triplet cube is nonzero only where the negative
index k is the same-label partner of the positive index j, i.e.
k = (j + N) mod 2N.  So the masked cube collapses to a [2N,2N] problem:

    t[i, j] = relu(d[i, j] - d[i, partner(j)] + 1)

All five reference outputs derive from t, the pairwise distances d, and
the row norms.  The anchor axis i (2N = 512 rows) is sharded across the
8 cores (64 rows each); each core computes its slab of the distance
matrix with PE matmuls (norm terms folded into the same PSUM
accumulation as rank-1 updates) and reduces its partial sums/counts to
4 scalars.  The host sums the 8 partials and assembles the outputs.

Per-core device outputs (res[4,1]):
    res[0] = sum of u over entries with u > 1e-5   (u = d0 - d1 + 1)
    res[1] = count of entries with u > 1e-5
    res[2] = count of entries with u < 1e-5
    res[3] = sum of squares of the whole batch (= sum of row norms^2)

Note relu never needs to be materialized: for eps = 1e-5 > 0,
{relu(u) > eps} == {u > eps}, {relu(u) < eps} == {u < eps}, and
relu(u)*(relu(u)>eps) == u*(u>eps).
"""

import os

import numpy as np

_TWO_N = 512  # 2N rows in the batch
_D = 512  # feature dim
_NCORES = 8
_S = _TWO_N // _NCORES  # 64 anchor rows per core
_KC = 128  # contraction chunk (partition dim)
_NK = _D // _KC  # 4 chunks
_EPS_REL = 1e-5

_NC_CACHE = None
LAST_RESULTS = None  # BassKernelResults of the most recent run (for profiling)


def _build_nc():
    import concourse.tile as tile
    from concourse import bacc, mybir

    f32 = mybir.dt.float32
    AF = mybir.ActivationFunctionType
    ALU = mybir.AluOpType
    AX = mybir.AxisListType

    nc = bacc.Bacc("TRN2", target_bir_lowering=False, debug=False)
    # bt[p, k, j]  = batch[j, k*128+p]          (batch.T, chunked over d)
    # stm2[p, k, i] = -2 * batch[r0+i, k*128+p] (this core's slab.T, scaled)
    bt_d = nc.dram_tensor("bt", [_KC, _NK, _TWO_N], f32, kind="ExternalInput")
    st_d = nc.dram_tensor("stm2", [_KC, _NK, _S], f32, kind="ExternalInput")
    res_d = nc.dram_tensor("res", [4, 1], f32, kind="ExternalOutput")

    with tile.TileContext(nc) as tc:
        with (
            tc.tile_pool(name="sb", bufs=1) as sb,
            tc.tile_pool(name="ps", bufs=1, space="PSUM") as ps,
        ):
            BT = sb.tile([_KC, _NK, _TWO_N], f32)
            nc.sync.dma_start(out=BT, in_=bt_d.ap())
            ST = sb.tile([_KC, _NK, _S], f32)
            nc.sync.dma_start(out=ST, in_=st_d.ap())

            ones_col = sb.tile([_KC, 1], f32)
            nc.vector.memset(ones_col, 1.0)
            ones_row = sb.tile([1, _TWO_N], f32)
            nc.vector.memset(ones_row, 1.0)

            BSQ = sb.tile([_KC, _NK, _TWO_N], f32)
            nc.vector.tensor_mul(BSQ, BT, BT)
            SSQ = sb.tile([_KC, _NK, _S], f32)
            nc.vector.tensor_mul(SSQ, ST, ST)  # = 4 * slab.T^2

            # n2row[j] = sum_d batch[j,d]^2, as a [1, 512] row in PSUM.
            n2row_ps = ps.tile([1, _TWO_N], f32)
            for k in range(_NK):
                nc.tensor.matmul(
                    n2row_ps,
                    lhsT=ones_col,
                    rhs=BSQ[:, k, :],
                    start=(k == 0),
                    stop=(k == _NK - 1),
                )
            # 4*n2slab[i] = sum_d (2*slab[i,d])^2, as a [1, 64] row.
            n2q_ps = ps.tile([1, _S], f32)
            for k in range(_NK):
                nc.tensor.matmul(
                    n2q_ps,
                    lhsT=ones_col,
                    rhs=SSQ[:, k, :],
                    start=(k == 0),
                    stop=(k == _NK - 1),
                )

            n2row_sb = sb.tile([1, _TWO_N], f32)
            nc.scalar.copy(n2row_sb, n2row_ps)
            n2i_sb = sb.tile([1, _S], f32)
            nc.scalar.mul(n2i_sb, n2q_ps, 0.25)

            # sq[i,j] = n2slab[i] + n2row[j] - 2*<slab_i, batch_j>, built
            # entirely inside one PSUM accumulation group: 4 contraction
            # chunks of (-2*slab.T).T @ batch.T plus two rank-1 updates.
            sq_ps = ps.tile([_S, _TWO_N], f32)
            for k in range(_NK):
                nc.tensor.matmul(
                    sq_ps,
                    lhsT=ST[:, k, :],
                    rhs=BT[:, k, :],
                    start=(k == 0),
                    stop=False,
                )
            nc.tensor.matmul(sq_ps, lhsT=n2i_sb, rhs=ones_row, start=False, stop=False)
            nc.tensor.matmul(
                sq_ps, lhsT=ones_row[:, 0:_S], rhs=n2row_sb, start=False, stop=True
            )

            # d = max(sqrt(max(sq, 1e-14)), 0) == clamp(sqrt-with-safe-zero)
            # of the reference: sqrt(max(sq, 1e-14)) equals
            # max(sqrt(relu(sq)), 1e-7) exactly in f32.
            sqc = sb.tile([_S, _TWO_N], f32)
            nc.vector.tensor_scalar_max(sqc, sq_ps, 1e-14)
            dmat = sb.tile([_S, _TWO_N], f32)
            nc.scalar.activation(dmat, sqc, AF.Sqrt)

            # u[i,j] = d[i,j] + 1 - d[i, partner(j)]; partner is a swap of
            # the two column halves.
            H = _TWO_N // 2
            u = sb.tile([_S, _TWO_N], f32)
            nc.vector.scalar_tensor_tensor(
                out=u[:, 0:H],
                in0=dmat[:, 0:H],
                scalar=1.0,
                in1=dmat[:, H:_TWO_N],
                op0=ALU.add,
                op1=ALU.subtract,
            )
            nc.vector.scalar_tensor_tensor(
                out=u[:, H:_TWO_N],
                in0=dmat[:, H:_TWO_N],
                scalar=1.0,
                in1=dmat[:, 0:H],
                op0=ALU.add,
                op1=ALU.subtract,
            )

            # Per-row partials land in red128[0:64, 0:3]; red128[:, 3] gets
            # the per-partition partial sums of BSQ (-> total sum of squares).
            red128 = sb.tile([_KC, 4], f32)
            nc.vector.memset(red128, 0.0)
            gt = sb.tile([_S, _TWO_N], f32)
            nc.vector.tensor_scalar(
                out=gt,
                in0=u,
                scalar1=_EPS_REL,
                scalar2=None,
                op0=ALU.is_gt,
                op1=ALU.add,
                accum_out=red128[0:_S, 1:2],
            )
            tsel = sb.tile([_S, _TWO_N], f32)
            nc.vector.scalar_tensor_tensor(
                out=tsel,
                in0=u,
                scalar=1.0,
                op0=ALU.mult,
                in1=gt,
                op1=ALU.mult,
                accum_out=red128[0:_S, 0:1],
            )
            lt = sb.tile([_S, _TWO_N], f32)
            nc.vector.tensor_scalar(
                out=lt,
                in0=u,
                scalar1=_EPS_REL,
                scalar2=None,
                op0=ALU.is_lt,
                op1=ALU.add,
                accum_out=red128[0:_S, 2:3],
            )
            nc.vector.reduce_sum(red128[:, 3:4], BSQ, axis=AX.XY)

            # Cross-partition sum of the four partial columns via PE.
            fin_ps = ps.tile([4, 1], f32)
            nc.tensor.matmul(fin_ps, lhsT=red128, rhs=ones_col, start=True, stop=True)
            fin_sb = sb.tile([4, 1], f32)
            nc.scalar.copy(fin_sb, fin_ps)
            nc.sync.dma_start(out=res_d.ap(), in_=fin_sb)

    nc.finalize()  # bacc register allocation + epilogue passes
    return nc


def _get_nc():
    global _NC_CACHE
    if _NC_CACHE is None:
        _NC_CACHE = _build_nc()
    return _NC_CACHE


def kernel(h1, h2, h3=None, **_unused):
    global LAST_RESULTS
    from concourse.bass_utils import run_bass_kernel_spmd

    h1 = np.ascontiguousarray(np.asarray(h1, dtype=np.float32))
    h2 = np.ascontiguousarray(np.asarray(h2, dtype=np.float32))
    batch = np.concatenate([h1, h2], axis=0)  # [2N, D]

    # bt[p, k, j] = batch[j, k*128+p]
    bt = np.ascontiguousarray(
        batch.T.reshape(_NK, _KC, _TWO_N).transpose(1, 0, 2)
    )
    in_maps = []
    for c in range(_NCORES):
        slab = batch[c * _S : (c + 1) * _S, :]  # [64, D]
        stm2 = np.ascontiguousarray(
            (-2.0 * slab).T.reshape(_NK, _KC, _S).transpose(1, 0, 2)
        )
        in_maps.append({"bt": bt, "stm2": stm2})

    trace = os.environ.get("BASS_TRIPLET_TRACE", "0") == "1"
    kw = {}
    if trace:
        kw["trace"] = True
        kw["trace_cores"] = [int(x) for x in
                             os.environ.get("BASS_TRIPLET_TRACE_CORES", "0").split(",")]
        tmpdir = os.environ.get("BASS_TRIPLET_TMPDIR")
        if tmpdir:
            kw["tmpdir"] = tmpdir

    res = run_bass_kernel_spmd(_get_nc(), in_maps, core_ids=list(range(_NCORES)), **kw)
    LAST_RESULTS = res

    sum_sel = 0.0
    cnt_gt = 0.0
    cnt_lt = 0.0
    for r in res.results:
        v = r["res"].reshape(4)
        sum_sel += float(v[0])
        cnt_gt += float(v[1])
        cnt_lt += float(v[2])
    sum_n2 = float(res.results[0]["res"].reshape(4)[3])

    mean_relevant = np.float32(sum_sel) / np.float32(cnt_gt)
    mean_norm_sq = np.float32(np.float32(sum_n2) / np.float32(_TWO_N))
    loss = np.float32(mean_relevant + np.float32(1e-4) * mean_norm_sq)
    mean_diff = np.float32(0.0)  # mean over the full antisymmetric cube is 0
    total = _TWO_N * _TWO_N * _TWO_N
    masked = _TWO_N * _TWO_N
    good = np.int32(total - masked + int(round(cnt_lt)))
    bad = np.int32(np.int32(total) - good)
    rms = np.float32(np.sqrt(mean_norm_sq))
    return (loss, mean_diff, good, bad, rms)


# revision 3
# speedup vs baseline: 1.0959x; 1.0959x over previous
"""BatchAllTripletLoss kernel for Trainium2 (8 NeuronCores, Bass/Tile).

Math shortcut: with labels = [0..N-1, 0..N-1], the (positive, negative)
mask of the [2N,2N,2N] triplet cube is nonzero only where the negative
index k is the same-label partner of the positive index j, i.e.
k = (j + N) mod 2N.  So the masked cube collapses to a [2N,2N] problem:

    t[i, j] = relu(d[i, j] - d[i, partner(j)] + 1)

All five reference outputs derive from t, the pairwise distances d, and
the row norms.  The anchor axis i (2N = 512 rows) is sharded across the
8 cores (64 rows each).

Per-core device pipeline (one PSUM accumulation holds the whole
squared-distance slab):

    sq[i,j] = n2[i] + n2[j] - 2<b_i, b_j>
    PSUM   += 4 chunk matmuls of (-2*slab.T).T @ batch.T      (the -2<,> term)
    PSUM   += ones[128,64].T @ (batch.T)^2-column-sums         (the n2[j] term:
              an all-ones lhsT sums BSQsum over partitions for every output row)
    n2[i]   comes in per-partition via the tensor_scalar epilogue (op0=add),
    which also applies max(.., 1e-14); sqrt then gives d exactly equal to
    the reference's clamp(where(sq>0, sqrt(sq), 0), 1e-7).

Reductions (per core, res[3,1]):
    res[0] = sum of u over entries with u > 1e-5   (u = d0 - d1 + 1)
    res[1] = count of entries with u > 1e-5
    res[2] = sum of squares of the whole batch

Host combine: count(u < eps) = 64*512*8 - count(u > eps) (no u can equal
f32(1e-5) exactly: u is produced by a subtraction at magnitude ~34, so its
value grid is multiples of 2^-19, which f32(1e-5) is not on), so
good = 2N^3 - CNT and bad = CNT.  relu never needs to be materialized:
for eps > 0, {relu(u) > eps} == {u > eps} and relu(u)*(..) == u*(u>eps).
mean(differences) over the full antisymmetric cube is exactly 0.
"""

import os

import numpy as np

_TWO_N = 512  # 2N rows in the batch
_D = 512  # feature dim
_NCORES = 8
_S = _TWO_N // _NCORES  # 64 anchor rows per core
_KC = 128  # contraction chunk (partition dim)
_NK = _D // _KC  # 4 chunks
_EPS_REL = 1e-5

_NC_CACHE = None
LAST_RESULTS = None  # BassKernelResults of the most recent run (for profiling)


def _build_nc():
    import concourse.tile as tile
    from concourse import bacc, mybir

    f32 = mybir.dt.float32
    AF = mybir.ActivationFunctionType
    ALU = mybir.AluOpType

    nc = bacc.Bacc("TRN2", target_bir_lowering=False, debug=False)
    # bt[p, k, j]  = batch[j, k*128+p]          (batch.T, chunked over d)
    # stm2[p, k, i] = -2 * batch[r0+i, k*128+p] (this core's slab.T, scaled)
    # sn[i, d]     = batch[r0+i, d]             (this core's slab, natural)
    bt_d = nc.dram_tensor("bt", [_KC, _NK, _TWO_N], f32, kind="ExternalInput")
    st_d = nc.dram_tensor("stm2", [_KC, _NK, _S], f32, kind="ExternalInput")
    sn_d = nc.dram_tensor("sn", [_S, _D], f32, kind="ExternalInput")
    res_d = nc.dram_tensor("res", [3, 1], f32, kind="ExternalOutput")

    with tile.TileContext(nc) as tc:
        with (
            tc.tile_pool(name="sb", bufs=1) as sb,
            tc.tile_pool(name="ps", bufs=1, space="PSUM") as ps,
        ):
            BT = sb.tile([_KC, _NK, _TWO_N], f32)
            for k in range(_NK):
                nc.sync.dma_start(out=BT[:, k, :], in_=bt_d.ap()[:, k, :])
            ST = sb.tile([_KC, _NK, _S], f32)
            nc.sync.dma_start(out=ST, in_=st_d.ap())
            SN = sb.tile([_S, _D], f32)
            nc.sync.dma_start(out=SN, in_=sn_d.ap())

            ones64 = sb.tile([_KC, _S], f32)
            nc.vector.memset(ones64, 1.0)
            ones_col = sb.tile([_KC, 1], f32)
            nc.vector.memset(ones_col, 1.0)
            red128 = sb.tile([_KC, 3], f32)
            nc.vector.memset(red128, 0.0)

            # n2slab[i] = sum_d slab[i,d]^2 as a per-partition scalar column.
            n2slab_col = sb.tile([_S, 1], f32)
            snsq = sb.tile([_S, _D], f32)
            nc.vector.scalar_tensor_tensor(
                out=snsq,
                in0=SN,
                scalar=1.0,
                op0=ALU.mult,
                in1=SN,
                op1=ALU.mult,
                accum_out=n2slab_col,
            )

            # BSQsum[p, j] = sum_k batch.T[k*128+p, j]^2; its partition sums
            # (via the all-ones matmul below) give n2[j].  The stt accum on
            # the last add simultaneously yields per-partition totals of the
            # whole batch's sum of squares -> res[2].
            BSQ = sb.tile([_KC, _NK, _TWO_N], f32)
            for k in range(_NK):
                nc.vector.tensor_mul(BSQ[:, k, :], BT[:, k, :], BT[:, k, :])
            BS01 = sb.tile([_KC, _TWO_N], f32)
            nc.vector.tensor_add(BS01, BSQ[:, 0, :], BSQ[:, 1, :])
            BS23 = sb.tile([_KC, _TWO_N], f32)
            nc.vector.tensor_add(BS23, BSQ[:, 2, :], BSQ[:, 3, :])
            BSQsum = sb.tile([_KC, _TWO_N], f32)
            nc.vector.scalar_tensor_tensor(
                out=BSQsum,
                in0=BS01,
                scalar=0.0,
                op0=ALU.add,
                in1=BS23,
                op1=ALU.add,
                accum_out=red128[:, 2:3],
            )

            # sq_ps[i,j] = -2<slab_i, b_j> + n2[j]
            sq_ps = ps.tile([_S, _TWO_N], f32)
            for k in range(_NK):
                nc.tensor.matmul(
                    sq_ps,
                    lhsT=ST[:, k, :],
                    rhs=BT[:, k, :],
                    start=(k == 0),
                    stop=False,
                )
            nc.tensor.matmul(sq_ps, lhsT=ones64, rhs=BSQsum, start=False, stop=True)

            # sqc = max(sq_ps + n2slab[i], 1e-14); d = sqrt(sqc) equals the
            # reference's max(sqrt(relu(sq)), 1e-7) exactly in f32.
            sqc = sb.tile([_S, _TWO_N], f32)
            nc.vector.tensor_scalar(
                out=sqc,
                in0=sq_ps,
                scalar1=n2slab_col,
                scalar2=1e-14,
                op0=ALU.add,
                op1=ALU.max,
            )
            dmat = sb.tile([_S, _TWO_N], f32)
            nc.scalar.activation(dmat, sqc, AF.Sqrt)

            # u[i,j] = d[i,j] + 1 - d[i, partner(j)]; partner swaps halves.
            H = _TWO_N // 2
            u = sb.tile([_S, _TWO_N], f32)
            nc.vector.scalar_tensor_tensor(
                out=u[:, 0:H],
                in0=dmat[:, 0:H],
                scalar=1.0,
                op0=ALU.add,
                in1=dmat[:, H:_TWO_N],
                op1=ALU.subtract,
            )
            nc.vector.scalar_tensor_tensor(
                out=u[:, H:_TWO_N],
                in0=dmat[:, H:_TWO_N],
                scalar=1.0,
                op0=ALU.add,
                in1=dmat[:, 0:H],
                op1=ALU.subtract,
            )

            gt = sb.tile([_S, _TWO_N], f32)
            nc.vector.tensor_scalar(
                out=gt,
                in0=u,
                scalar1=_EPS_REL,
                scalar2=None,
                op0=ALU.is_gt,
                op1=ALU.add,
                accum_out=red128[0:_S, 1:2],
            )
            tsel = sb.tile([_S, _TWO_N], f32)
            nc.vector.scalar_tensor_tensor(
                out=tsel,
                in0=u,
                scalar=1.0,
                op0=ALU.mult,
                in1=gt,
                op1=ALU.mult,
                accum_out=red128[0:_S, 0:1],
            )

            # Cross-partition sum of the three partial columns via PE.
            fin_ps = ps.tile([3, 1], f32)
            nc.tensor.matmul(fin_ps, lhsT=red128, rhs=ones_col, start=True, stop=True)
            fin_sb = sb.tile([3, 1], f32)
            nc.vector.tensor_copy(fin_sb, fin_ps)
            nc.sync.dma_start(out=res_d.ap(), in_=fin_sb)

    nc.finalize()  # bacc register allocation + epilogue passes
    return nc


def _get_nc():
    global _NC_CACHE
    if _NC_CACHE is None:
        _NC_CACHE = _build_nc()
    return _NC_CACHE


def kernel(h1, h2, h3=None, **_unused):
    global LAST_RESULTS
    from concourse.bass_utils import run_bass_kernel_spmd

    h1 = np.ascontiguousarray(np.asarray(h1, dtype=np.float32))
    h2 = np.ascontiguousarray(np.asarray(h2, dtype=np.float32))
    batch = np.concatenate([h1, h2], axis=0)  # [2N, D]

    # bt[p, k, j] = batch[j, k*128+p]
    bt = np.ascontiguousarray(batch.T.reshape(_NK, _KC, _TWO_N).transpose(1, 0, 2))
    in_maps = []
    for c in range(_NCORES):
        slab = batch[c * _S : (c + 1) * _S, :]  # [64, D]
        stm2 = np.ascontiguousarray(
            (-2.0 * slab).T.reshape(_NK, _KC, _S).transpose(1, 0, 2)
        )
        in_maps.append(
            {"bt": bt, "stm2": stm2, "sn": np.ascontiguousarray(slab)}
        )

    trace = os.environ.get("BASS_TRIPLET_TRACE", "0") == "1"
    kw = {}
    if trace:
        kw["trace"] = True
        kw["trace_cores"] = [
            int(x)
            for x in os.environ.get("BASS_TRIPLET_TRACE_CORES", "0").split(",")
        ]
        tmpdir = os.environ.get("BASS_TRIPLET_TMPDIR")
        if tmpdir:
            kw["tmpdir"] = tmpdir

    res = run_bass_kernel_spmd(_get_nc(), in_maps, core_ids=list(range(_NCORES)), **kw)
    LAST_RESULTS = res

    sum_sel = 0.0
    cnt_gt = 0.0
    for r in res.results:
        v = r["res"].reshape(3)
        sum_sel += float(v[0])
        cnt_gt += float(v[1])
    sum_n2 = float(res.results[0]["res"].reshape(3)[2])

    mean_relevant = np.float32(sum_sel) / np.float32(cnt_gt)
    mean_norm_sq = np.float32(np.float32(sum_n2) / np.float32(_TWO_N))
    loss = np.float32(mean_relevant + np.float32(1e-4) * mean_norm_sq)
    mean_diff = np.float32(0.0)  # mean over the full antisymmetric cube is 0
    total = _TWO_N * _TWO_N * _TWO_N
    cnt_i = int(round(cnt_gt))
    good = np.int32(total - cnt_i)
    bad = np.int32(cnt_i)
    rms = np.float32(np.sqrt(mean_norm_sq))
    return (loss, mean_diff, good, bad, rms)


# revision 6
# speedup vs baseline: 1.1468x; 1.0465x over previous
"""BatchAllTripletLoss kernel for Trainium2 (8 NeuronCores, Bass/Tile).

Math shortcut: with labels = [0..N-1, 0..N-1], the (positive, negative)
mask of the [2N,2N,2N] triplet cube is nonzero only where the negative
index k is the same-label partner of the positive index j, i.e.
k = (j + N) mod 2N.  So the masked cube collapses to a [2N,2N] problem:

    t[i, j] = relu(d[i, j] - d[i, partner(j)] + 1)

All five reference outputs derive from t, the pairwise distances d, and
the row norms.  The anchor axis i (2N = 512 rows) is sharded across the
8 cores (64 rows each).

Per-core device pipeline (one PSUM accumulation holds the whole
squared-distance slab):

    sq[i,j] = n2[i] + n2[j] - 2<b_i, b_j>
    PSUM   += 4 chunk matmuls of (-2*slab.T).T @ batch.T      (the -2<,> term)
    PSUM   += ones[128,64].T @ (batch.T)^2-column-sums         (the n2[j] term:
              an all-ones lhsT sums BSQsum over partitions for every output row)
    n2[i]   comes in per-partition via the tensor_scalar epilogue (op0=add),
    which also applies max(.., 1e-14); sqrt then gives d exactly equal to
    the reference's clamp(where(sq>0, sqrt(sq), 0), 1e-7).

Reductions (per core, res[3,1]):
    res[0] = sum of u over entries with u > 1e-5   (u = d0 - d1 + 1)
    res[1] = count of entries with u > 1e-5
    res[2] = sum of squares of the whole batch

Host combine: count(u < eps) = 64*512*8 - count(u > eps) (no u can equal
f32(1e-5) exactly: u is produced by a subtraction at magnitude ~34, so its
value grid is multiples of 2^-19, which f32(1e-5) is not on), so
good = 2N^3 - CNT and bad = CNT.  relu never needs to be materialized:
for eps > 0, {relu(u) > eps} == {u > eps} and relu(u)*(..) == u*(u>eps).
mean(differences) over the full antisymmetric cube is exactly 0.
"""

import os

import numpy as np

_TWO_N = 512  # 2N rows in the batch
_D = 512  # feature dim
_NCORES = 8
_S = _TWO_N // _NCORES  # 64 anchor rows per core
_KC = 128  # contraction chunk (partition dim)
_NK = _D // _KC  # 4 chunks
_EPS_REL = 1e-5

_NC_CACHE = None
LAST_RESULTS = None  # BassKernelResults of the most recent run (for profiling)


def _build_nc():
    import concourse.tile as tile
    from concourse import bacc, mybir

    f32 = mybir.dt.float32
    AF = mybir.ActivationFunctionType
    ALU = mybir.AluOpType

    nc = bacc.Bacc("TRN2", target_bir_lowering=False, debug=False)
    # bt[p, k, j]  = batch[j, k*128+p]          (batch.T, chunked over d)
    # stm2[p, k, i] = -2 * batch[r0+i, k*128+p] (this core's slab.T, scaled)
    # sn[i, d]     = batch[r0+i, d]             (this core's slab, natural)
    bt_d = nc.dram_tensor("bt", [_KC, _NK, _TWO_N], f32, kind="ExternalInput")
    st_d = nc.dram_tensor("stm2", [_KC, _NK, _S], f32, kind="ExternalInput")
    sn_d = nc.dram_tensor("sn", [_S, _D], f32, kind="ExternalInput")
    res_d = nc.dram_tensor("res", [3, 1], f32, kind="ExternalOutput")

    with tile.TileContext(nc) as tc:
        with (
            tc.tile_pool(name="sb", bufs=1) as sb,
            tc.tile_pool(name="ps", bufs=1, space="PSUM") as ps,
        ):
            # Small, critical inputs first (PE's first matmul needs the whole
            # ST tile); bt chunks follow, split across the two HWDGE rings
            # (sync=SP, scalar=Act) so descriptor issue isn't serialized.
            ST = sb.tile([_KC, _NK, _S], f32)
            nc.sync.dma_start(out=ST, in_=st_d.ap())
            SN = sb.tile([_S, _D], f32)
            nc.scalar.dma_start(out=SN, in_=sn_d.ap())
            BT = sb.tile([_KC, _NK, _TWO_N], f32)
            dma_engines = [nc.sync, nc.scalar, nc.sync, nc.scalar]
            for k in range(_NK):
                dma_engines[k].dma_start(out=BT[:, k, :], in_=bt_d.ap()[:, k, :])

            ones64 = sb.tile([_KC, _S], f32)
            nc.vector.memset(ones64, 1.0)
            ones_col = sb.tile([_KC, 1], f32)
            nc.vector.memset(ones_col, 1.0)
            red128 = sb.tile([_KC, 3], f32)
            nc.vector.memset(red128, 0.0)

            # n2slab[i] = sum_d slab[i,d]^2 as a per-partition scalar column.
            n2slab_col = sb.tile([_S, 1], f32)
            snsq = sb.tile([_S, _D], f32)
            nc.vector.scalar_tensor_tensor(
                out=snsq,
                in0=SN,
                scalar=1.0,
                op0=ALU.mult,
                in1=SN,
                op1=ALU.mult,
                accum_out=n2slab_col,
            )

            # BSQsum[p, j] = sum_k batch.T[k*128+p, j]^2; its partition sums
            # (via the all-ones matmul below) give n2[j].  The stt accum on
            # the last add simultaneously yields per-partition totals of the
            # whole batch's sum of squares -> res[2].
            BSQ = sb.tile([_KC, _NK, _TWO_N], f32)
            for k in range(_NK):
                nc.vector.tensor_mul(BSQ[:, k, :], BT[:, k, :], BT[:, k, :])
            BS01 = sb.tile([_KC, _TWO_N], f32)
            nc.vector.tensor_add(BS01, BSQ[:, 0, :], BSQ[:, 1, :])
            BS23 = sb.tile([_KC, _TWO_N], f32)
            nc.vector.tensor_add(BS23, BSQ[:, 2, :], BSQ[:, 3, :])
            BSQsum = sb.tile([_KC, _TWO_N], f32)
            nc.vector.scalar_tensor_tensor(
                out=BSQsum,
                in0=BS01,
                scalar=0.0,
                op0=ALU.add,
                in1=BS23,
                op1=ALU.add,
                accum_out=red128[:, 2:3],
            )

            # sq_ps[i,j] = -2<slab_i, b_j> + n2[j]
            sq_ps = ps.tile([_S, _TWO_N], f32)
            for k in range(_NK):
                nc.tensor.matmul(
                    sq_ps,
                    lhsT=ST[:, k, :],
                    rhs=BT[:, k, :],
                    start=(k == 0),
                    stop=False,
                )
            nc.tensor.matmul(sq_ps, lhsT=ones64, rhs=BSQsum, start=False, stop=True)

            # sqc = max(sq_ps + n2slab[i], 1e-14); d = sqrt(sqc) equals the
            # reference's max(sqrt(relu(sq)), 1e-7) exactly in f32.
            sqc = sb.tile([_S, _TWO_N], f32)
            nc.vector.tensor_scalar(
                out=sqc,
                in0=sq_ps,
                scalar1=n2slab_col,
                scalar2=1e-14,
                op0=ALU.add,
                op1=ALU.max,
            )
            dmat = sb.tile([_S, _TWO_N], f32)
            nc.scalar.activation(dmat, sqc, AF.Sqrt)

            # u[i,j] = d[i,j] + 1 - d[i, partner(j)]; partner swaps halves.
            H = _TWO_N // 2
            u = sb.tile([_S, _TWO_N], f32)
            nc.vector.scalar_tensor_tensor(
                out=u[:, 0:H],
                in0=dmat[:, 0:H],
                scalar=1.0,
                op0=ALU.add,
                in1=dmat[:, H:_TWO_N],
                op1=ALU.subtract,
            )
            nc.vector.scalar_tensor_tensor(
                out=u[:, H:_TWO_N],
                in0=dmat[:, H:_TWO_N],
                scalar=1.0,
                op0=ALU.add,
                in1=dmat[:, 0:H],
                op1=ALU.subtract,
            )

            gt = sb.tile([_S, _TWO_N], f32)
            nc.vector.tensor_scalar(
                out=gt,
                in0=u,
                scalar1=_EPS_REL,
                scalar2=None,
                op0=ALU.is_gt,
                op1=ALU.add,
                accum_out=red128[0:_S, 1:2],
            )
            tsel = sb.tile([_S, _TWO_N], f32)
            nc.vector.scalar_tensor_tensor(
                out=tsel,
                in0=u,
                scalar=1.0,
                op0=ALU.mult,
                in1=gt,
                op1=ALU.mult,
                accum_out=red128[0:_S, 0:1],
            )

            # Cross-partition sum of the three partial columns via PE.
            fin_ps = ps.tile([3, 1], f32)
            nc.tensor.matmul(fin_ps, lhsT=red128, rhs=ones_col, start=True, stop=True)
            fin_sb = sb.tile([3, 1], f32)
            nc.vector.tensor_copy(fin_sb, fin_ps)
            nc.sync.dma_start(out=res_d.ap(), in_=fin_sb)

    nc.finalize()  # bacc register allocation + epilogue passes
    return nc


def _get_nc():
    global _NC_CACHE
    if _NC_CACHE is None:
        _NC_CACHE = _build_nc()
    return _NC_CACHE


def kernel(h1, h2, h3=None, **_unused):
    global LAST_RESULTS
    from concourse.bass_utils import run_bass_kernel_spmd

    h1 = np.ascontiguousarray(np.asarray(h1, dtype=np.float32))
    h2 = np.ascontiguousarray(np.asarray(h2, dtype=np.float32))
    batch = np.concatenate([h1, h2], axis=0)  # [2N, D]

    # bt[p, k, j] = batch[j, k*128+p]
    bt = np.ascontiguousarray(batch.T.reshape(_NK, _KC, _TWO_N).transpose(1, 0, 2))
    in_maps = []
    for c in range(_NCORES):
        slab = batch[c * _S : (c + 1) * _S, :]  # [64, D]
        stm2 = np.ascontiguousarray(
            (-2.0 * slab).T.reshape(_NK, _KC, _S).transpose(1, 0, 2)
        )
        in_maps.append(
            {"bt": bt, "stm2": stm2, "sn": np.ascontiguousarray(slab)}
        )

    trace = os.environ.get("BASS_TRIPLET_TRACE", "0") == "1"
    kw = {}
    if trace:
        kw["trace"] = True
        kw["trace_cores"] = [
            int(x)
            for x in os.environ.get("BASS_TRIPLET_TRACE_CORES", "0").split(",")
        ]
        tmpdir = os.environ.get("BASS_TRIPLET_TMPDIR")
        if tmpdir:
            kw["tmpdir"] = tmpdir

    res = run_bass_kernel_spmd(_get_nc(), in_maps, core_ids=list(range(_NCORES)), **kw)
    LAST_RESULTS = res

    sum_sel = 0.0
    cnt_gt = 0.0
    for r in res.results:
        v = r["res"].reshape(3)
        sum_sel += float(v[0])
        cnt_gt += float(v[1])
    sum_n2 = float(res.results[0]["res"].reshape(3)[2])

    mean_relevant = np.float32(sum_sel) / np.float32(cnt_gt)
    mean_norm_sq = np.float32(np.float32(sum_n2) / np.float32(_TWO_N))
    loss = np.float32(mean_relevant + np.float32(1e-4) * mean_norm_sq)
    mean_diff = np.float32(0.0)  # mean over the full antisymmetric cube is 0
    total = _TWO_N * _TWO_N * _TWO_N
    cnt_i = int(round(cnt_gt))
    good = np.int32(total - cnt_i)
    bad = np.int32(cnt_i)
    rms = np.float32(np.sqrt(mean_norm_sq))
    return (loss, mean_diff, good, bad, rms)
